# revision 1
# baseline (speedup 1.0000x reference)
"""Bass/Trainium2 kernel for nn_HMEClassification (hierarchical mixture-of-experts).

Strategy: pure data parallel across 8 cores (batch sharded). Per core:
  xT [128d, 16384b] streamed in 512-wide b-tiles (bf16).
  L1 (7 units: 3 gates + 4 experts): weight-stationary bf16 matmuls
      lhsT=W1 block [128d,128h], rhs=xT tile [128d,512b] -> PSUM [128h,512b],
      fp32 accumulate. Evacuated PSUM->SBUF bf16 with fused bias+relu on
      ScalarE/VectorE.
  L2 experts: col-tiled pairs, lhsT=eW2 chunk [128h,64c], K-accumulated over
      4 h-chunks -> PSUM [128(=2x64c), 512b] logits^T.
  Gates: softmax over 2 == sigmoid(z0-z1) with difference weights. Two PSUM
      banks laid out so DVE ops stay partition-aligned:
        psG1 rows {0,1}=d_root,d_root  rows {32,33}=-d_root,-d_root
        psG2 rows {0,1}=d_A,-d_A       rows {32,33}=d_B,-d_B
      sigmoid -> T1=(rA,rA|rB,rB), T2=(gA0,gA1|gB0,gB1); P = T1*T2 gives the
      four root*gate products at rows {0,1,32,33}.
  Softmax over classes: exp on ScalarE (logits are O(1), no max needed),
      partition sums via ones-select matmul (S at rows {0,1,32,33}),
      C = P/S, DMA partition-broadcast of C rows to 64-partition blocks,
      prod = exp * C, final 4-expert sum via stacked-identity matmul.
  Output out^T [64, 16384] fp32 per core; host transposes/concats.
"""

import ml_dtypes
import numpy as np

import concourse.bass as bass
import concourse.mybir as mybir
import concourse.tile as tile
from concourse import bacc
from concourse.bass_utils import run_bass_kernel_spmd

B, D, H, C = 131072, 128, 512, 64
NCORES = 8
BC = B // NCORES        # 16384 rows per core
TB = 512                # b-tile width
KH = H // 128           # 4 h-chunks of 128

F32 = mybir.dt.float32
BF16 = mybir.dt.bfloat16

# ---- bf16 consts layout (columns in [128, NB] bf16 tensor) ----
W1_OFF = 0                       # 7 units * 512 = 3584
W2_OFF = W1_OFF + 7 * H          # 16 blocks (k*4+e) * 64 = 1024
GP_OFF = W2_OFF + 16 * 64        # 4 chunks * 8 = 32
OS_OFF = GP_OFF + 32             # 2 cols (ones select)
ID_OFF = OS_OFF + 2              # 64 cols (stacked identity)
NB = ID_OFF + 64
# ---- fp32 consts layout ----
B1_OFF = 0                       # 28 cols (u*4+hb)
EB_OFF = B1_OFF + 28             # 2 cols
GB_OFF = EB_OFF + 2              # 2 cols (sigmoid biases for T1, T2)
NF = GB_OFF + 2


def _build_consts(gW1, gb1, gW2, gb2, eW1, eb1, eW2, eb2):
    cb = np.zeros((128, NB), dtype=np.float32)
    for u in range(3):
        cb[:, W1_OFF + u * H: W1_OFF + (u + 1) * H] = gW1[u]
    for e in range(4):
        cb[:, W1_OFF + (3 + e) * H: W1_OFF + (4 + e) * H] = eW1[e]
    for k in range(KH):
        for e in range(4):
            cb[:, W2_OFF + (k * 4 + e) * 64: W2_OFF + (k * 4 + e + 1) * 64] = \
                eW2[e, k * 128:(k + 1) * 128, :]
    v = gW2[:, :, 0] - gW2[:, :, 1]          # [3, 512]
    for k in range(KH):
        sl = slice(k * 128, (k + 1) * 128)
        blk = np.stack([v[0, sl], v[0, sl], -v[0, sl], -v[0, sl],
                        v[1, sl], -v[1, sl], v[2, sl], -v[2, sl]], axis=1)
        cb[:, GP_OFF + k * 8: GP_OFF + (k + 1) * 8] = blk
    cb[:64, OS_OFF + 0] = 1.0
    cb[64:, OS_OFF + 1] = 1.0
    p = np.arange(128)
    cb[:, ID_OFF: ID_OFF + 64] = (p[:, None] % 64 == np.arange(64)[None, :])

    cf = np.zeros((128, NF), dtype=np.float32)
    b1 = np.concatenate([gb1, eb1], axis=0)  # [7, 512]
    for u in range(7):
        for hb in range(KH):
            cf[:, B1_OFF + u * 4 + hb] = b1[u, hb * 128:(hb + 1) * 128]
    cf[:64, EB_OFF + 0] = eb2[0]
    cf[64:, EB_OFF + 0] = eb2[1]
    cf[:64, EB_OFF + 1] = eb2[2]
    cf[64:, EB_OFF + 1] = eb2[3]
    db = gb2[:, 0] - gb2[:, 1]               # [3]
    cf[0:2, GB_OFF] = db[0]
    cf[32:34, GB_OFF] = -db[0]
    cf[0:2, GB_OFF + 1] = [db[1], -db[1]]
    cf[32:34, GB_OFF + 1] = [db[2], -db[2]]
    return cb.astype(ml_dtypes.bfloat16), cf


def _bcast_src(scr, pair):
    """DRAM AP streaming rows (2p x64, 2p+1 x64) of scratch [4, TB]: matches a
    [128, TB] SBUF destination partition-major."""
    row = scr[2 * pair: 2 * pair + 1, :]
    return bass.AP(tensor=row.tensor, offset=row.offset,
                   ap=[[TB, 2], [0, 64], [1, TB]])


def _build_nc(n_tiles):
    nc = bacc.Bacc("TRN2", target_bir_lowering=False)
    xt = nc.dram_tensor("xt", [D, BC], BF16, kind="ExternalInput")
    cbd = nc.dram_tensor("cb", [128, NB], BF16, kind="ExternalInput")
    cfd = nc.dram_tensor("cf", [128, NF], F32, kind="ExternalInput")
    outT = nc.dram_tensor("outT", [C, BC], F32, kind="ExternalOutput")

    AF = mybir.ActivationFunctionType
    OP = mybir.AluOpType

    with tile.TileContext(nc) as tc:
        with (
            tc.tile_pool(name="singles", bufs=1) as singles,
            tc.tile_pool(name="xp", bufs=3) as xp,
            tc.tile_pool(name="hp", bufs=2) as hp,
            tc.tile_pool(name="ep", bufs=2) as ep,
            tc.tile_pool(name="sp", bufs=2) as sp,
            tc.tile_pool(name="op", bufs=2) as op_pool,
            tc.tile_pool(name="psL1", bufs=3, space="PSUM") as psL1p,
            tc.tile_pool(name="psE", bufs=1, space="PSUM") as psEp,
            tc.tile_pool(name="psG", bufs=1, space="PSUM") as psGp,
            tc.tile_pool(name="psS", bufs=1, space="PSUM") as psSp,
            tc.tile_pool(name="psO", bufs=1, space="PSUM") as psOp,
            tc.tile_pool(name="drp", bufs=2, space="DRAM") as drp,
        ):
            cs = singles.tile([128, NB], BF16)
            nc.sync.dma_start(out=cs, in_=cbd[:, :])
            cf = singles.tile([128, NF], F32)
            nc.sync.dma_start(out=cf, in_=cfd[:, :])

            def w1_ap(u, hb):
                a = W1_OFF + u * H + hb * 128
                return cs[:, a: a + 128]

            def w2_ap(k, e):
                a = W2_OFF + (k * 4 + e) * 64
                return cs[:, a: a + 64]

            def gp_ap(k, j):
                a = GP_OFF + k * 8 + j * 2
                return cs[:, a: a + 2]

            for t in range(n_tiles):
                xtile = xp.tile([D, TB], BF16, tag="x")
                nc.sync.dma_start(out=xtile, in_=xt[:, t * TB:(t + 1) * TB])

                # ---- L1: 7 units x 4 h-blocks ----
                hsb = {}
                for u in range(7):
                    for hb in range(KH):
                        ps = psL1p.tile([128, TB], F32, tag="l1")
                        nc.tensor.matmul(ps, w1_ap(u, hb), xtile,
                                         start=True, stop=True)
                        h = hp.tile([128, TB], BF16, tag=f"h{u}_{hb}", bufs=2)
                        bias_ap = cf[:, B1_OFF + u * 4 + hb: B1_OFF + u * 4 + hb + 1]
                        idx = u * KH + hb
                        if idx % 5 < 3:
                            nc.scalar.activation(h, ps, AF.Relu, bias=bias_ap)
                        else:
                            nc.vector.tensor_scalar(h, ps, bias_ap, 0.0,
                                                    op0=OP.add, op1=OP.max)
                        hsb[u, hb] = h

                # ---- L2 experts: pairs (e0,e1) and (e2,e3), col-tiled ----
                expc = ep.tile([128, 2 * TB], BF16, tag="exp")
                for pair in range(2):
                    psE = psEp.tile([128, TB], F32, tag="e2")
                    ua, ub = 3 + 2 * pair, 4 + 2 * pair
                    for k in range(KH):
                        nc.tensor.matmul(psE[0:64, :], w2_ap(k, 2 * pair),
                                         hsb[ua, k], start=(k == 0),
                                         stop=(k == KH - 1),
                                         tile_position=(0, 0))
                        nc.tensor.matmul(psE[64:128, :], w2_ap(k, 2 * pair + 1),
                                         hsb[ub, k], start=(k == 0),
                                         stop=(k == KH - 1),
                                         tile_position=(0, 64))
                    eb_ap = cf[:, EB_OFF + pair: EB_OFF + pair + 1]
                    nc.scalar.activation(expc[:, pair * TB:(pair + 1) * TB],
                                         psE, AF.Exp, bias=eb_ap)

                # ---- gates: two banks, aligned rows {0,1} and {32,33} ----
                psG1 = psGp.tile([34, TB], F32, tag="g1")
                psG2 = psGp.tile([34, TB], F32, tag="g2")
                for k in range(KH):
                    st, sp_ = (k == 0), (k == KH - 1)
                    nc.tensor.matmul(psG1[0:2, :], gp_ap(k, 0), hsb[0, k],
                                     start=st, stop=sp_, tile_position=(0, 0))
                    nc.tensor.matmul(psG1[32:34, :], gp_ap(k, 1), hsb[0, k],
                                     start=st, stop=sp_, tile_position=(0, 32))
                    nc.tensor.matmul(psG2[0:2, :], gp_ap(k, 2), hsb[1, k],
                                     start=st, stop=sp_, tile_position=(0, 0))
                    nc.tensor.matmul(psG2[32:34, :], gp_ap(k, 3), hsb[2, k],
                                     start=st, stop=sp_, tile_position=(0, 32))
                T1 = sp.tile([34, TB], F32, tag="T1")
                T2 = sp.tile([34, TB], F32, tag="T2")
                nc.scalar.activation(T1, psG1, AF.Sigmoid,
                                     bias=cf[0:34, GB_OFF: GB_OFF + 1])
                nc.scalar.activation(T2, psG2, AF.Sigmoid,
                                     bias=cf[0:34, GB_OFF + 1: GB_OFF + 2])

                # ---- partition sums of exp via ones-select matmul ----
                psS = psSp.tile([34, TB], F32, tag="s")
                nc.tensor.matmul(psS[0:2, :], cs[:, OS_OFF: OS_OFF + 2],
                                 expc[:, 0:TB], start=True, stop=True,
                                 tile_position=(0, 0))
                nc.tensor.matmul(psS[32:34, :], cs[:, OS_OFF: OS_OFF + 2],
                                 expc[:, TB: 2 * TB], start=True, stop=True,
                                 tile_position=(0, 32))

                # ---- combine coeffs C = (root*gate)/S at rows {0,1,32,33} ----
                P = sp.tile([34, TB], F32, tag="P")
                nc.vector.tensor_tensor(P, T1, T2, op=OP.mult)
                U = sp.tile([34, TB], F32, tag="U")
                nc.vector.reciprocal(U[0:2, :], psS[0:2, :])
                nc.vector.reciprocal(U[32:34, :], psS[32:34, :])
                Cf_t = sp.tile([34, TB], BF16, tag="C")
                nc.vector.tensor_tensor(Cf_t[0:2, :], P[0:2, :], U[0:2, :],
                                        op=OP.mult)
                nc.vector.tensor_tensor(Cf_t[32:34, :], P[32:34, :],
                                        U[32:34, :], op=OP.mult)

                # ---- DMA partition-broadcast of coeff rows via DRAM scratch ----
                scr = drp.tile([4, TB], BF16, tag="scr")
                nc.sync.dma_start(out=scr[0:2, :], in_=Cf_t[0:2, :])
                nc.sync.dma_start(out=scr[2:4, :], in_=Cf_t[32:34, :])
                prods = []
                for pair in range(2):
                    cbc = sp.tile([128, TB], BF16, tag=f"cbc{pair}")
                    nc.sync.dma_start(out=cbc, in_=_bcast_src(scr, pair))
                    prod = sp.tile([128, TB], BF16, tag=f"prod{pair}")
                    nc.vector.tensor_tensor(
                        prod, expc[:, pair * TB:(pair + 1) * TB], cbc,
                        op=OP.mult)
                    prods.append(prod)

                # ---- final sum of 4 experts via stacked identity ----
                psO = psOp.tile([64, TB], F32, tag="o")
                id2 = cs[:, ID_OFF: ID_OFF + 64]
                nc.tensor.matmul(psO, id2, prods[0], start=True, stop=False)
                nc.tensor.matmul(psO, id2, prods[1], start=False, stop=True)
                osb = op_pool.tile([64, TB], F32, tag="osb")
                nc.scalar.copy(osb, psO)
                nc.sync.dma_start(out=outT[:, t * TB:(t + 1) * TB], in_=osb)

    nc.compile()
    return nc


def kernel(x, gW1, gb1, gW2, gb2, eW1, eb1, eW2, eb2, _trace=False):
    x = np.asarray(x, dtype=np.float32)
    cb, cf = _build_consts(
        np.asarray(gW1, np.float32), np.asarray(gb1, np.float32),
        np.asarray(gW2, np.float32), np.asarray(gb2, np.float32),
        np.asarray(eW1, np.float32), np.asarray(eb1, np.float32),
        np.asarray(eW2, np.float32), np.asarray(eb2, np.float32))
    n_rows = x.shape[0]
    bc = n_rows // NCORES
    n_tiles = bc // TB
    assert bc * NCORES == n_rows and n_tiles * TB == bc

    global BC
    BC = bc
    nc = _build_nc(n_tiles)

    xs = x.reshape(NCORES, bc, D)
    in_maps = [
        {"xt": np.ascontiguousarray(xs[c].T).astype(ml_dtypes.bfloat16),
         "cb": cb, "cf": cf}
        for c in range(NCORES)
    ]
    res = run_bass_kernel_spmd(nc, in_maps, core_ids=list(range(NCORES)),
                               trace=_trace)
    out = np.concatenate([r["outT"].T for r in res.results], axis=0)
    kernel.last_results = res
    return np.ascontiguousarray(out.astype(np.float32))



# revision 9
# speedup vs baseline: 1.5113x; 1.5113x over previous
"""Bass/Trainium2 kernel for nn_HMEClassification (hierarchical MoE), v2.

Data parallel across 8 cores (batch sharded). Per core xT [128d, 16384b],
processed in 512-wide b-tiles.

Per tile (TB=512):
  L1 (7 units x 4 h-blocks): weight-stationary bf16 MMs -> PSUM pairs
      [128, 1024] (2 banks), evacuated with fused bias+relu -> bf16 hsb.
      Experts (u=3..6) first so L2 can start early; gates (0..2) after.
  L2 experts: per k-chunk, expert pair MMs at tile_position (0,0)/(0,64)
      (concurrent col-groups), K-accumulated -> psE [128, 1024] (pair0|pair1).
      Exp evac with class-bias -> expc bf16.
  Gates (log-domain, single ACT table set 'natural_log_exp_and_others'):
      psG bank [68, 512]: G1 (0,0) rows 0-31 = (z0,-z0,0..); G2 (0,32) rows
      32-63 = (zA,-zA,zB,-zB,0..); ones-select S MMs (0,64) rows 64-67 =
      per-expert softmax denominators. In-place E=exp(-z-db) on rows 0-63,
      then ONE Ln over rows 0-68 with per-partition bias (+1 rows 0-63, +0
      rows 64-67) gives V = [-ln sigma terms | ln S] in SBUF f32.
  Coefficient broadcast via f32r matmul: psB[p,b] = sum_q M[q,p]*V[q,b]
      with M in {0,-1} -> delta = ln(root*gate/S) per 64-row block; cbc =
      exp(psB) (ACT); prod = expc*cbc (DVE/GpSimd); final sum over experts
      via stacked-identity MMs -> psO [64, 512]; copy -> DMA out.
"""

import ml_dtypes
import numpy as np

import concourse.bass as bass
import concourse.mybir as mybir
import concourse.tile as tile
from concourse import bacc
from concourse.bass_utils import run_bass_kernel_spmd

B, D, H, C = 131072, 128, 512, 64
NCORES = 8
TB = 512                # b-tile width
KH = H // 128           # 4 h-chunks of 128

F32 = mybir.dt.float32
F32R = mybir.dt.float32r
BF16 = mybir.dt.bfloat16

# ---- bf16 consts layout (columns in [128, NB] bf16 tensor) ----
W1_OFF = 0                       # 7 units * 512 = 3584
W2_OFF = W1_OFF + 7 * H          # 16 blocks (k*4+e) * 64 = 1024
G1_OFF = W2_OFF + 16 * 64        # 4 k * 32 cols (d0,-d0,0..)
G2A_OFF = G1_OFF + 4 * 32        # 4 k * 32 cols (dA,-dA,0..)
G2B_OFF = G2A_OFF + 4 * 32       # 4 k * 32 cols (0,0,dB,-dB,0..)
SA_OFF = G2B_OFF + 4 * 32        # 4 cols ones-select pair0
SB_OFF = SA_OFF + 4              # 4 cols ones-select pair1
ID_OFF = SB_OFF + 4              # 64 cols stacked identity
NB = ID_OFF + 64
# ---- fp32 consts layout ----
B1_OFF = 0                       # 28 cols (u*4+hb) L1 biases
EB_OFF = B1_OFF + 28             # 2 cols expert class biases (pair0, pair1)
GB_OFF = EB_OFF + 2              # 1 col gate-bias for E-exp (rows 0-35)
LB_OFF = GB_OFF + 1              # 1 col Ln bias (1.0 rows 0-63, 0.0 rows 64+)
NF = LB_OFF + 1
# ---- f32 M matrix [68, 256]: two 128-col blocks (pair0, pair1) ----


def _build_consts(gW1, gb1, gW2, gb2, eW1, eb1, eW2, eb2):
    cb = np.zeros((128, NB), dtype=np.float32)
    for u in range(3):
        cb[:, W1_OFF + u * H: W1_OFF + (u + 1) * H] = gW1[u]
    for e in range(4):
        cb[:, W1_OFF + (3 + e) * H: W1_OFF + (4 + e) * H] = eW1[e]
    for k in range(KH):
        for e in range(4):
            cb[:, W2_OFF + (k * 4 + e) * 64: W2_OFF + (k * 4 + e + 1) * 64] = \
                eW2[e, k * 128:(k + 1) * 128, :]
    v = gW2[:, :, 0] - gW2[:, :, 1]          # [3, 512]
    for k in range(KH):
        sl = slice(k * 128, (k + 1) * 128)
        cb[:, G1_OFF + k * 32 + 0] = v[0, sl]
        cb[:, G1_OFF + k * 32 + 1] = -v[0, sl]
        cb[:, G2A_OFF + k * 32 + 0] = v[1, sl]
        cb[:, G2A_OFF + k * 32 + 1] = -v[1, sl]
        cb[:, G2B_OFF + k * 32 + 2] = v[2, sl]
        cb[:, G2B_OFF + k * 32 + 3] = -v[2, sl]
    cb[0:64, SA_OFF + 0] = 1.0
    cb[64:128, SA_OFF + 1] = 1.0
    cb[0:64, SB_OFF + 2] = 1.0
    cb[64:128, SB_OFF + 3] = 1.0
    p = np.arange(128)
    cb[:, ID_OFF: ID_OFF + 64] = (p[:, None] % 64 == np.arange(64)[None, :])

    cf = np.zeros((128, NF), dtype=np.float32)
    b1 = np.concatenate([gb1, eb1], axis=0)  # [7, 512]
    for u in range(7):
        for hb in range(KH):
            cf[:, B1_OFF + u * 4 + hb] = b1[u, hb * 128:(hb + 1) * 128]
    cf[:64, EB_OFF + 0] = eb2[0]
    cf[64:, EB_OFF + 0] = eb2[1]
    cf[:64, EB_OFF + 1] = eb2[2]
    cf[64:, EB_OFF + 1] = eb2[3]
    db = gb2[:, 0] - gb2[:, 1]               # [3]
    # E-exp bias: E = exp(-(z + db)) = exp(-z + bias), bias rows:
    #   row 0: -db0 (z0), row 1: +db0 (-z0), rows 32/33: -+dbA, 34/35: -+dbB
    gB = np.zeros(128, dtype=np.float32)
    gB[0], gB[1] = -db[0], db[0]
    gB[32], gB[33] = -db[1], db[1]
    gB[34], gB[35] = -db[2], db[2]
    cf[:, GB_OFF] = gB
    lb = np.zeros(128, dtype=np.float32)
    lb[0:64] = 1.0                           # ln(E + 1) rows; ln(S + 0) rows
    cf[:, LB_OFF] = lb

    # M [68, 256] f32: delta = sum_q M[q, p] * V[q, b]
    M = np.zeros((68, 256), dtype=np.float32)
    # pair0 (cols 0-127): A1 block rows {0, 32, 64}; A2 block {0, 33, 65}
    M[0, 0:64] = -1.0; M[32, 0:64] = -1.0; M[64, 0:64] = -1.0
    M[0, 64:128] = -1.0; M[33, 64:128] = -1.0; M[65, 64:128] = -1.0
    # pair1 (cols 128-255): B1 {1, 34, 66}; B2 {1, 35, 67}
    M[1, 128:192] = -1.0; M[34, 128:192] = -1.0; M[66, 128:192] = -1.0
    M[1, 192:256] = -1.0; M[35, 192:256] = -1.0; M[67, 192:256] = -1.0

    zero_b1 = not (np.any(gb1) or np.any(eb1))
    return cb.astype(ml_dtypes.bfloat16), cf, M, zero_b1


def _build_nc(n_tiles, bc, zero_b1):
    nc = bacc.Bacc("TRN2", target_bir_lowering=False)
    xt = nc.dram_tensor("xt", [D, bc], BF16, kind="ExternalInput")
    cbd = nc.dram_tensor("cb", [128, NB], BF16, kind="ExternalInput")
    cfd = nc.dram_tensor("cf", [128, NF], F32, kind="ExternalInput")
    md = nc.dram_tensor("md", [68, 256], F32R, kind="ExternalInput")
    outT = nc.dram_tensor("outT", [C, bc], F32, kind="ExternalOutput")

    AF = mybir.ActivationFunctionType
    OP = mybir.AluOpType

    # expert units first so L2 can start early; gate units last
    U_ORDER = [3, 4, 5, 6, 0, 1, 2]

    with tile.TileContext(nc) as tc:
        with (
            tc.tile_pool(name="singles", bufs=1) as singles,
            tc.tile_pool(name="xp", bufs=3) as xp,
            tc.tile_pool(name="hp", bufs=2) as hp,
            tc.tile_pool(name="ep", bufs=2) as ep,
            tc.tile_pool(name="vp", bufs=2) as vp,
            tc.tile_pool(name="cp", bufs=2) as cp,
            tc.tile_pool(name="pp", bufs=2) as pp,
            tc.tile_pool(name="op", bufs=2) as op_pool,
            tc.tile_pool(name="psL", bufs=2, space="PSUM") as psLp,
            tc.tile_pool(name="psE", bufs=1, space="PSUM") as psEp,
            tc.tile_pool(name="psG", bufs=1, space="PSUM") as psGp,
            tc.tile_pool(name="psT", bufs=1, space="PSUM") as psTp,
        ):
            cs = singles.tile([128, NB], BF16)
            nc.sync.dma_start(out=cs, in_=cbd[:, :])
            cf = singles.tile([128, NF], F32)
            nc.sync.dma_start(out=cf, in_=cfd[:, :])
            mm_ = singles.tile([68, 256], F32R)
            nc.sync.dma_start(out=mm_, in_=md[:, :])

            def w1_ap(u, hb):
                a = W1_OFF + u * H + hb * 128
                return cs[:, a: a + 128]

            def w2_ap(k, e):
                a = W2_OFF + (k * 4 + e) * 64
                return cs[:, a: a + 64]

            for t in range(n_tiles):
                xtile = xp.tile([D, TB], BF16, tag="x")
                nc.sync.dma_start(out=xtile, in_=xt[:, t * TB:(t + 1) * TB])

                # ---- L1: 7 units x 4 h-blocks, paired into 2-bank PSUM ----
                hsb = {}
                evac_alt = 0
                for ui, u in enumerate(U_ORDER):
                    for hb2 in range(0, KH, 2):
                        ps = psLp.tile([128, 2 * TB], F32, tag="l1")
                        nc.tensor.matmul(ps[:, 0:TB], w1_ap(u, hb2), xtile,
                                         start=True, stop=True)
                        nc.tensor.matmul(ps[:, TB:2 * TB], w1_ap(u, hb2 + 1),
                                         xtile, start=True, stop=True)
                        h = hp.tile([128, 2 * TB], BF16,
                                    tag=f"h{u}_{hb2}", bufs=2)
                        if zero_b1:
                            # wide single-op evac, no bias
                            if evac_alt % 7 < 3:
                                nc.scalar.activation(h, ps, AF.Relu)
                            else:
                                nc.vector.tensor_scalar(
                                    h, ps, 0.0, None, op0=OP.max)
                        else:
                            for j in range(2):
                                bap = cf[:, B1_OFF + u * 4 + hb2 + j:
                                         B1_OFF + u * 4 + hb2 + j + 1]
                                hj = h[:, j * TB:(j + 1) * TB]
                                pj = ps[:, j * TB:(j + 1) * TB]
                                if evac_alt % 7 < 3:
                                    nc.scalar.activation(hj, pj, AF.Relu,
                                                         bias=bap)
                                else:
                                    nc.vector.tensor_scalar(
                                        hj, pj, bap, 0.0,
                                        op0=OP.add, op1=OP.max)
                        evac_alt += 1
                        hsb[u, hb2] = h

                def h_ap(u, k):
                    base = hsb[u, (k // 2) * 2]
                    j = k % 2
                    return base[:, j * TB:(j + 1) * TB]

                # ---- L2 experts: K-accumulated pairs, e/e+1 concurrent ----
                psE = psEp.tile([128, 2 * TB], F32, tag="e2")
                for k in range(KH):
                    for pair in range(2):
                        sl = slice(pair * TB, (pair + 1) * TB)
                        nc.tensor.matmul(psE[0:64, sl], w2_ap(k, 2 * pair),
                                         h_ap(3 + 2 * pair, k),
                                         start=(k == 0), stop=(k == KH - 1),
                                         tile_position=(0, 0))
                        nc.tensor.matmul(psE[64:128, sl],
                                         w2_ap(k, 2 * pair + 1),
                                         h_ap(4 + 2 * pair, k),
                                         start=(k == 0), stop=(k == KH - 1),
                                         tile_position=(0, 64))
                expc = ep.tile([128, 2 * TB], BF16, tag="exp")
                for pair in range(2):
                    sl = slice(pair * TB, (pair + 1) * TB)
                    nc.scalar.activation(
                        expc[:, sl], psE[:, sl], AF.Exp,
                        bias=cf[:, EB_OFF + pair: EB_OFF + pair + 1])

                # ---- gates + S in one PSUM bank [68, TB] ----
                psG = psGp.tile([68, TB], F32, tag="g")
                for k in range(KH):
                    nc.tensor.matmul(psG[0:32, :],
                                     cs[:, G1_OFF + k * 32: G1_OFF + k * 32 + 32],
                                     h_ap(0, k), start=(k == 0),
                                     stop=(k == KH - 1), tile_position=(0, 0))
                for k in range(KH):
                    nc.tensor.matmul(psG[32:64, :],
                                     cs[:, G2A_OFF + k * 32: G2A_OFF + k * 32 + 32],
                                     h_ap(1, k), start=(k == 0), stop=False,
                                     tile_position=(0, 32))
                    nc.tensor.matmul(psG[32:64, :],
                                     cs[:, G2B_OFF + k * 32: G2B_OFF + k * 32 + 32],
                                     h_ap(2, k), start=False,
                                     stop=(k == KH - 1),
                                     tile_position=(0, 32))
                # S rows 64-67: ones-select sums of expc halves
                nc.tensor.matmul(psG[64:68, :], cs[:, SA_OFF: SA_OFF + 4],
                                 expc[:, 0:TB], start=True, stop=False,
                                 tile_position=(0, 64))
                nc.tensor.matmul(psG[64:68, :], cs[:, SB_OFF: SB_OFF + 4],
                                 expc[:, TB:2 * TB], start=False, stop=True,
                                 tile_position=(0, 64))

                # ---- E = exp(-z - db) in place; V = Ln(E + 1 | S + 0) ----
                nc.scalar.activation(psG[0:64, :], psG[0:64, :], AF.Exp,
                                     bias=cf[0:64, GB_OFF: GB_OFF + 1],
                                     scale=-1.0)
                V = vp.tile([68, TB], F32R, tag="v")
                nc.scalar.activation(V, psG[0:68, :], AF.Ln,
                                     bias=cf[0:68, LB_OFF: LB_OFF + 1])

                # ---- delta broadcast via f32r MM; cbc = exp(delta) ----
                prods = []
                for pair in range(2):
                    psB = psTp.tile([128, TB], F32, tag="tail")
                    nc.tensor.matmul(
                        psB, mm_[:, pair * 128:(pair + 1) * 128],
                        V[:, :], start=True, stop=True)
                    cbc = cp.tile([128, TB], BF16, tag=f"c{pair}")
                    nc.scalar.activation(cbc, psB, AF.Exp)
                    prod = pp.tile([128, TB], BF16, tag=f"p{pair}")
                    nc.vector.tensor_tensor(
                        prod, expc[:, pair * TB:(pair + 1) * TB], cbc,
                        op=OP.mult)
                    prods.append(prod)

                # ---- final: stacked-identity sum over 4 experts ----
                psO = psTp.tile([128, TB], F32, tag="tail")
                id2 = cs[:, ID_OFF: ID_OFF + 64]
                nc.tensor.matmul(psO[0:64, :], id2, prods[0],
                                 start=True, stop=False)
                nc.tensor.matmul(psO[0:64, :], id2, prods[1],
                                 start=False, stop=True)
                osb = op_pool.tile([64, TB], F32, tag="osb")
                nc.vector.tensor_copy(osb, psO[0:64, :])
                nc.sync.dma_start(out=outT[:, t * TB:(t + 1) * TB], in_=osb)

    nc.compile()
    return nc


def kernel(x, gW1, gb1, gW2, gb2, eW1, eb1, eW2, eb2, _trace=False):
    x = np.asarray(x, dtype=np.float32)
    cb, cf, M, zero_b1 = _build_consts(
        np.asarray(gW1, np.float32), np.asarray(gb1, np.float32),
        np.asarray(gW2, np.float32), np.asarray(gb2, np.float32),
        np.asarray(eW1, np.float32), np.asarray(eb1, np.float32),
        np.asarray(eW2, np.float32), np.asarray(eb2, np.float32))
    n_rows = x.shape[0]
    bc = n_rows // NCORES
    n_tiles = bc // TB
    assert bc * NCORES == n_rows and n_tiles * TB == bc

    nc = _build_nc(n_tiles, bc, zero_b1)

    xs = x.reshape(NCORES, bc, D)
    in_maps = [
        {"xt": np.ascontiguousarray(xs[c].T).astype(ml_dtypes.bfloat16),
         "cb": cb, "cf": cf, "md": M}
        for c in range(NCORES)
    ]
    res = run_bass_kernel_spmd(nc, in_maps, core_ids=list(range(NCORES)),
                               trace=_trace)
    out = np.concatenate([r["outT"].T for r in res.results], axis=0)
    kernel.last_results = res
    return np.ascontiguousarray(out.astype(np.float32))


# revision 12
# speedup vs baseline: 1.8257x; 1.2081x over previous
"""Bass/Trainium2 kernel for nn_HMEClassification (hierarchical MoE), v2.

Data parallel across 8 cores (batch sharded). Per core xT [128d, 16384b],
processed in 512-wide b-tiles.

Per tile (TB=512):
  L1 (7 units x 4 h-blocks): weight-stationary bf16 MMs -> PSUM pairs
      [128, 1024] (2 banks), evacuated with fused bias+relu -> bf16 hsb.
      Experts (u=3..6) first so L2 can start early; gates (0..2) after.
  L2 experts: per k-chunk, expert pair MMs at tile_position (0,0)/(0,64)
      (concurrent col-groups), K-accumulated -> psE [128, 1024] (pair0|pair1).
      Exp evac with class-bias -> expc bf16.
  Gates (log-domain, single ACT table set 'natural_log_exp_and_others'):
      psG bank [68, 512]: G1 (0,0) rows 0-31 = (z0,-z0,0..); G2 (0,32) rows
      32-63 = (zA,-zA,zB,-zB,0..); ones-select S MMs (0,64) rows 64-67 =
      per-expert softmax denominators. In-place E=exp(-z-db) on rows 0-63,
      then ONE Ln over rows 0-68 with per-partition bias (+1 rows 0-63, +0
      rows 64-67) gives V = [-ln sigma terms | ln S] in SBUF f32.
  Coefficient broadcast via f32r matmul: psB[p,b] = sum_q M[q,p]*V[q,b]
      with M in {0,-1} -> delta = ln(root*gate/S) per 64-row block; cbc =
      exp(psB) (ACT); prod = expc*cbc (DVE/GpSimd); final sum over experts
      via stacked-identity MMs -> psO [64, 512]; copy -> DMA out.
"""

import json
import os
import tempfile

import ml_dtypes
import numpy as np

import concourse.bass as bass
import concourse.mybir as mybir
import concourse.tile as tile
from concourse import bacc
from concourse.bass_utils import run_bass_kernel_spmd


def _setup_act_tables():
    """Reorder act_info.json so 'natural_log_exp_and_others' is first: the
    table-set chooser picks the first set containing each function, so Exp
    and Ln then share one resident table set (no per-tile ACT_TABLE_LOAD
    thrash). Points both bacc (python) and walrus (--act-root-json) at the
    same reordered copy so set indices agree."""
    from neuronxcc.driver.Job import Job
    from neuronxcc.driver.jobs.support.FindActInfo import findActInfoFile
    src = findActInfoFile(Job.getPackageDir(), "gen3")
    src_dir = os.path.dirname(src)
    dst_dir = os.path.join(tempfile.gettempdir(), "pwp_nle_first")
    os.makedirs(dst_dir, exist_ok=True)
    for f in os.listdir(src_dir):
        link = os.path.join(dst_dir, f)
        if f != "act_info.json" and not os.path.exists(link):
            os.symlink(os.path.join(src_dir, f), link)
    info = json.load(open(src))
    sets = info["act_func_sets"]
    sets.sort(key=lambda e: e["name"] != "natural_log_exp_and_others")
    dst = os.path.join(dst_dir, "act_info.json")
    with open(dst, "w") as f:
        json.dump(info, f)
    os.environ["BASS_ACT_ROOT_JSON_PATH"] = dst

    import concourse.hw_specs as hw_specs
    tables = {
        ent["name"]: {
            mybir.ActivationFunctionType.from_pwp(v)
            for v in ent["act"].keys()
        }
        for ent in info["act_func_sets"]
    }
    bacc.get_activation_tables = lambda arch: tables
    hw_specs.get_activation_tables = lambda arch: tables

B, D, H, C = 131072, 128, 512, 64
NCORES = 8
TB = 512                # b-tile width
KH = H // 128           # 4 h-chunks of 128

F32 = mybir.dt.float32
F32R = mybir.dt.float32r
BF16 = mybir.dt.bfloat16

# ---- bf16 consts layout (columns in [128, NB] bf16 tensor) ----
W1_OFF = 0                       # 7 units * 512 = 3584
W2_OFF = W1_OFF + 7 * H          # 16 blocks (k*4+e) * 64 = 1024
G1_OFF = W2_OFF + 16 * 64        # 4 k * 32 cols (d0,-d0,0..)
G2A_OFF = G1_OFF + 4 * 32        # 4 k * 32 cols (dA,-dA,0..)
G2B_OFF = G2A_OFF + 4 * 32       # 4 k * 32 cols (0,0,dB,-dB,0..)
SA_OFF = G2B_OFF + 4 * 32        # 4 cols ones-select pair0
SB_OFF = SA_OFF + 4              # 4 cols ones-select pair1
ID_OFF = SB_OFF + 4              # 64 cols stacked identity
NB = ID_OFF + 64
# ---- fp32 consts layout ----
B1_OFF = 0                       # 28 cols (u*4+hb) L1 biases
EB_OFF = B1_OFF + 28             # 2 cols expert class biases (pair0, pair1)
GB_OFF = EB_OFF + 2              # 1 col gate-bias for E-exp (rows 0-35)
LB_OFF = GB_OFF + 1              # 1 col Ln bias (1.0 rows 0-63, 0.0 rows 64+)
NF = LB_OFF + 1
# ---- f32 M matrix [68, 256]: two 128-col blocks (pair0, pair1) ----


def _build_consts(gW1, gb1, gW2, gb2, eW1, eb1, eW2, eb2):
    cb = np.zeros((128, NB), dtype=np.float32)
    for u in range(3):
        cb[:, W1_OFF + u * H: W1_OFF + (u + 1) * H] = gW1[u]
    for e in range(4):
        cb[:, W1_OFF + (3 + e) * H: W1_OFF + (4 + e) * H] = eW1[e]
    for k in range(KH):
        for e in range(4):
            cb[:, W2_OFF + (k * 4 + e) * 64: W2_OFF + (k * 4 + e + 1) * 64] = \
                eW2[e, k * 128:(k + 1) * 128, :]
    v = gW2[:, :, 0] - gW2[:, :, 1]          # [3, 512]
    for k in range(KH):
        sl = slice(k * 128, (k + 1) * 128)
        cb[:, G1_OFF + k * 32 + 0] = v[0, sl]
        cb[:, G1_OFF + k * 32 + 1] = -v[0, sl]
        cb[:, G2A_OFF + k * 32 + 0] = v[1, sl]
        cb[:, G2A_OFF + k * 32 + 1] = -v[1, sl]
        cb[:, G2B_OFF + k * 32 + 2] = v[2, sl]
        cb[:, G2B_OFF + k * 32 + 3] = -v[2, sl]
    cb[0:64, SA_OFF + 0] = 1.0
    cb[64:128, SA_OFF + 1] = 1.0
    cb[0:64, SB_OFF + 2] = 1.0
    cb[64:128, SB_OFF + 3] = 1.0
    p = np.arange(128)
    cb[:, ID_OFF: ID_OFF + 64] = (p[:, None] % 64 == np.arange(64)[None, :])

    cf = np.zeros((128, NF), dtype=np.float32)
    b1 = np.concatenate([gb1, eb1], axis=0)  # [7, 512]
    for u in range(7):
        for hb in range(KH):
            cf[:, B1_OFF + u * 4 + hb] = b1[u, hb * 128:(hb + 1) * 128]
    cf[:64, EB_OFF + 0] = eb2[0]
    cf[64:, EB_OFF + 0] = eb2[1]
    cf[:64, EB_OFF + 1] = eb2[2]
    cf[64:, EB_OFF + 1] = eb2[3]
    db = gb2[:, 0] - gb2[:, 1]               # [3]
    # E-exp bias: E = exp(-(z + db)) = exp(-z + bias), bias rows:
    #   row 0: -db0 (z0), row 1: +db0 (-z0), rows 32/33: -+dbA, 34/35: -+dbB
    gB = np.zeros(128, dtype=np.float32)
    gB[0], gB[1] = -db[0], db[0]
    gB[32], gB[33] = -db[1], db[1]
    gB[34], gB[35] = -db[2], db[2]
    cf[:, GB_OFF] = gB
    lb = np.zeros(128, dtype=np.float32)
    lb[0:64] = 1.0                           # ln(E + 1) rows; ln(S + 0) rows
    cf[:, LB_OFF] = lb

    # M [68, 256] f32: delta = sum_q M[q, p] * V[q, b]
    M = np.zeros((68, 256), dtype=np.float32)
    # pair0 (cols 0-127): A1 block rows {0, 32, 64}; A2 block {0, 33, 65}
    M[0, 0:64] = -1.0; M[32, 0:64] = -1.0; M[64, 0:64] = -1.0
    M[0, 64:128] = -1.0; M[33, 64:128] = -1.0; M[65, 64:128] = -1.0
    # pair1 (cols 128-255): B1 {1, 34, 66}; B2 {1, 35, 67}
    M[1, 128:192] = -1.0; M[34, 128:192] = -1.0; M[66, 128:192] = -1.0
    M[1, 192:256] = -1.0; M[35, 192:256] = -1.0; M[67, 192:256] = -1.0

    zero_b1 = not (np.any(gb1) or np.any(eb1))
    return cb.astype(ml_dtypes.bfloat16), cf, M, zero_b1


def _build_nc(n_tiles, bc, zero_b1):
    _setup_act_tables()
    nc = bacc.Bacc("TRN2", target_bir_lowering=False)
    xt = nc.dram_tensor("xt", [D, bc], BF16, kind="ExternalInput")
    cbd = nc.dram_tensor("cb", [128, NB], BF16, kind="ExternalInput")
    cfd = nc.dram_tensor("cf", [128, NF], F32, kind="ExternalInput")
    md = nc.dram_tensor("md", [68, 256], F32R, kind="ExternalInput")
    outT = nc.dram_tensor("outT", [C, bc], F32, kind="ExternalOutput")

    AF = mybir.ActivationFunctionType
    OP = mybir.AluOpType

    # expert units first so L2 can start early; gate units last
    U_ORDER = [3, 4, 5, 6, 0, 1, 2]

    with tile.TileContext(nc) as tc:
        with (
            tc.tile_pool(name="singles", bufs=1) as singles,
            tc.tile_pool(name="xp", bufs=3) as xp,
            tc.tile_pool(name="hp", bufs=3) as hp,
            tc.tile_pool(name="ep", bufs=2) as ep,
            tc.tile_pool(name="vp", bufs=2) as vp,
            tc.tile_pool(name="cp", bufs=2) as cp,
            tc.tile_pool(name="pp", bufs=2) as pp,
            tc.tile_pool(name="op", bufs=2) as op_pool,
            tc.tile_pool(name="psL", bufs=2, space="PSUM") as psLp,
            tc.tile_pool(name="psE", bufs=1, space="PSUM") as psEp,
            tc.tile_pool(name="psG", bufs=1, space="PSUM") as psGp,
            tc.tile_pool(name="psT", bufs=1, space="PSUM") as psTp,
        ):
            cs = singles.tile([128, NB], BF16)
            nc.sync.dma_start(out=cs, in_=cbd[:, :])
            cf = singles.tile([128, NF], F32)
            nc.sync.dma_start(out=cf, in_=cfd[:, :])
            mm_ = singles.tile([68, 256], F32R)
            nc.sync.dma_start(out=mm_, in_=md[:, :])

            def w1_ap(u, hb):
                a = W1_OFF + u * H + hb * 128
                return cs[:, a: a + 128]

            def w2_ap(k, e):
                a = W2_OFF + (k * 4 + e) * 64
                return cs[:, a: a + 64]

            for t in range(n_tiles):
                xtile = xp.tile([D, TB], BF16, tag="x")
                nc.sync.dma_start(out=xtile, in_=xt[:, t * TB:(t + 1) * TB])

                # ---- L1: 7 units x 4 h-blocks, paired into 2-bank PSUM ----
                hsb = {}
                evac_alt = 0
                for ui, u in enumerate(U_ORDER):
                    for hb2 in range(0, KH, 2):
                        ps = psLp.tile([128, 2 * TB], F32, tag="l1")
                        nc.tensor.matmul(ps[:, 0:TB], w1_ap(u, hb2), xtile,
                                         start=True, stop=True)
                        nc.tensor.matmul(ps[:, TB:2 * TB], w1_ap(u, hb2 + 1),
                                         xtile, start=True, stop=True)
                        h = hp.tile([128, 2 * TB], BF16,
                                    tag=f"h{u}_{hb2}", bufs=2)
                        if zero_b1:
                            # wide single-op evac, no bias
                            if evac_alt % 7 < 3:
                                nc.scalar.activation(h, ps, AF.Relu)
                            else:
                                nc.vector.tensor_scalar(
                                    h, ps, 0.0, None, op0=OP.max)
                        else:
                            for j in range(2):
                                bap = cf[:, B1_OFF + u * 4 + hb2 + j:
                                         B1_OFF + u * 4 + hb2 + j + 1]
                                hj = h[:, j * TB:(j + 1) * TB]
                                pj = ps[:, j * TB:(j + 1) * TB]
                                if evac_alt % 7 < 3:
                                    nc.scalar.activation(hj, pj, AF.Relu,
                                                         bias=bap)
                                else:
                                    nc.vector.tensor_scalar(
                                        hj, pj, bap, 0.0,
                                        op0=OP.add, op1=OP.max)
                        evac_alt += 1
                        hsb[u, hb2] = h

                def h_ap(u, k):
                    base = hsb[u, (k // 2) * 2]
                    j = k % 2
                    return base[:, j * TB:(j + 1) * TB]

                # ---- L2 experts: K-accumulated pairs, e/e+1 concurrent ----
                psE = psEp.tile([128, 2 * TB], F32, tag="e2")
                for k in range(KH):
                    for pair in range(2):
                        sl = slice(pair * TB, (pair + 1) * TB)
                        nc.tensor.matmul(psE[0:64, sl], w2_ap(k, 2 * pair),
                                         h_ap(3 + 2 * pair, k),
                                         start=(k == 0), stop=(k == KH - 1),
                                         tile_position=(0, 0))
                        nc.tensor.matmul(psE[64:128, sl],
                                         w2_ap(k, 2 * pair + 1),
                                         h_ap(4 + 2 * pair, k),
                                         start=(k == 0), stop=(k == KH - 1),
                                         tile_position=(0, 64))
                expc = ep.tile([128, 2 * TB], BF16, tag="exp")
                for pair in range(2):
                    sl = slice(pair * TB, (pair + 1) * TB)
                    nc.scalar.activation(
                        expc[:, sl], psE[:, sl], AF.Exp,
                        bias=cf[:, EB_OFF + pair: EB_OFF + pair + 1])

                # ---- gates + S in one PSUM bank [68, TB] ----
                psG = psGp.tile([68, TB], F32, tag="g")
                for k in range(KH):
                    nc.tensor.matmul(psG[0:32, :],
                                     cs[:, G1_OFF + k * 32: G1_OFF + k * 32 + 32],
                                     h_ap(0, k), start=(k == 0),
                                     stop=(k == KH - 1), tile_position=(0, 0))
                for k in range(KH):
                    nc.tensor.matmul(psG[32:64, :],
                                     cs[:, G2A_OFF + k * 32: G2A_OFF + k * 32 + 32],
                                     h_ap(1, k), start=(k == 0), stop=False,
                                     tile_position=(0, 32))
                    nc.tensor.matmul(psG[32:64, :],
                                     cs[:, G2B_OFF + k * 32: G2B_OFF + k * 32 + 32],
                                     h_ap(2, k), start=False,
                                     stop=(k == KH - 1),
                                     tile_position=(0, 32))
                # S rows 64-67: ones-select sums of expc halves
                nc.tensor.matmul(psG[64:68, :], cs[:, SA_OFF: SA_OFF + 4],
                                 expc[:, 0:TB], start=True, stop=False,
                                 tile_position=(0, 64))
                nc.tensor.matmul(psG[64:68, :], cs[:, SB_OFF: SB_OFF + 4],
                                 expc[:, TB:2 * TB], start=False, stop=True,
                                 tile_position=(0, 64))

                # ---- E = exp(-z - db) in place; V = Ln(E + 1 | S + 0) ----
                nc.scalar.activation(psG[0:64, :], psG[0:64, :], AF.Exp,
                                     bias=cf[0:64, GB_OFF: GB_OFF + 1],
                                     scale=-1.0)
                V = vp.tile([68, TB], F32R, tag="v")
                nc.scalar.activation(V, psG[0:68, :], AF.Ln,
                                     bias=cf[0:68, LB_OFF: LB_OFF + 1])

                # ---- delta broadcast via f32r MM; cbc = exp(delta) ----
                prods = []
                for pair in range(2):
                    psB = psTp.tile([128, TB], F32, tag="tail")
                    nc.tensor.matmul(
                        psB, mm_[:, pair * 128:(pair + 1) * 128],
                        V[:, :], start=True, stop=True)
                    cbc = cp.tile([128, TB], BF16, tag=f"c{pair}")
                    nc.scalar.activation(cbc, psB, AF.Exp)
                    prod = pp.tile([128, TB], BF16, tag=f"p{pair}")
                    nc.vector.tensor_tensor(
                        prod, expc[:, pair * TB:(pair + 1) * TB], cbc,
                        op=OP.mult)
                    prods.append(prod)

                # ---- final: stacked-identity sum over 4 experts ----
                psO = psTp.tile([128, TB], F32, tag="tail")
                id2 = cs[:, ID_OFF: ID_OFF + 64]
                nc.tensor.matmul(psO[0:64, :], id2, prods[0],
                                 start=True, stop=False)
                nc.tensor.matmul(psO[0:64, :], id2, prods[1],
                                 start=False, stop=True)
                osb = op_pool.tile([64, TB], F32, tag="osb")
                nc.vector.tensor_copy(osb, psO[0:64, :])
                nc.sync.dma_start(out=outT[:, t * TB:(t + 1) * TB], in_=osb)

    nc.compile()
    return nc


def kernel(x, gW1, gb1, gW2, gb2, eW1, eb1, eW2, eb2, _trace=False):
    x = np.asarray(x, dtype=np.float32)
    cb, cf, M, zero_b1 = _build_consts(
        np.asarray(gW1, np.float32), np.asarray(gb1, np.float32),
        np.asarray(gW2, np.float32), np.asarray(gb2, np.float32),
        np.asarray(eW1, np.float32), np.asarray(eb1, np.float32),
        np.asarray(eW2, np.float32), np.asarray(eb2, np.float32))
    n_rows = x.shape[0]
    bc = n_rows // NCORES
    n_tiles = bc // TB
    assert bc * NCORES == n_rows and n_tiles * TB == bc

    nc = _build_nc(n_tiles, bc, zero_b1)

    xs = x.reshape(NCORES, bc, D)
    in_maps = [
        {"xt": np.ascontiguousarray(xs[c].T).astype(ml_dtypes.bfloat16),
         "cb": cb, "cf": cf, "md": M}
        for c in range(NCORES)
    ]
    res = run_bass_kernel_spmd(nc, in_maps, core_ids=list(range(NCORES)),
                               trace=_trace)
    out = np.concatenate([r["outT"].T for r in res.results], axis=0)
    kernel.last_results = res
    return np.ascontiguousarray(out.astype(np.float32))


# revision 13
# speedup vs baseline: 2.1247x; 1.1638x over previous
"""Bass/Trainium2 kernel for nn_HMEClassification (hierarchical MoE), v2.

Data parallel across 8 cores (batch sharded). Per core xT [128d, 16384b],
processed in 512-wide b-tiles.

Per tile (TB=512):
  L1 (7 units x 4 h-blocks): weight-stationary bf16 MMs -> PSUM pairs
      [128, 1024] (2 banks), evacuated with fused bias+relu -> bf16 hsb.
      Experts (u=3..6) first so L2 can start early; gates (0..2) after.
  L2 experts: per k-chunk, expert pair MMs at tile_position (0,0)/(0,64)
      (concurrent col-groups), K-accumulated -> psE [128, 1024] (pair0|pair1).
      Exp evac with class-bias -> expc bf16.
  Gates (log-domain, single ACT table set 'natural_log_exp_and_others'):
      psG bank [68, 512]: G1 (0,0) rows 0-31 = (z0,-z0,0..); G2 (0,32) rows
      32-63 = (zA,-zA,zB,-zB,0..); ones-select S MMs (0,64) rows 64-67 =
      per-expert softmax denominators. In-place E=exp(-z-db) on rows 0-63,
      then ONE Ln over rows 0-68 with per-partition bias (+1 rows 0-63, +0
      rows 64-67) gives V = [-ln sigma terms | ln S] in SBUF f32.
  Coefficient broadcast via f32r matmul: psB[p,b] = sum_q M[q,p]*V[q,b]
      with M in {0,-1} -> delta = ln(root*gate/S) per 64-row block; cbc =
      exp(psB) (ACT); prod = expc*cbc (DVE/GpSimd); final sum over experts
      via stacked-identity MMs -> psO [64, 512]; copy -> DMA out.
"""

import json
import os
import tempfile

import ml_dtypes
import numpy as np

import concourse.bass as bass
import concourse.mybir as mybir
import concourse.tile as tile
from concourse import bacc
from concourse.bass_utils import run_bass_kernel_spmd


def _setup_act_tables():
    """Reorder act_info.json so 'natural_log_exp_and_others' is first: the
    table-set chooser picks the first set containing each function, so Exp
    and Ln then share one resident table set (no per-tile ACT_TABLE_LOAD
    thrash). Points both bacc (python) and walrus (--act-root-json) at the
    same reordered copy so set indices agree."""
    from neuronxcc.driver.Job import Job
    from neuronxcc.driver.jobs.support.FindActInfo import findActInfoFile
    src = findActInfoFile(Job.getPackageDir(), "gen3")
    src_dir = os.path.dirname(src)
    dst_dir = os.path.join(tempfile.gettempdir(), "pwp_nle_first")
    os.makedirs(dst_dir, exist_ok=True)
    for f in os.listdir(src_dir):
        link = os.path.join(dst_dir, f)
        if f != "act_info.json" and not os.path.exists(link):
            os.symlink(os.path.join(src_dir, f), link)
    info = json.load(open(src))
    sets = info["act_func_sets"]
    sets.sort(key=lambda e: e["name"] != "natural_log_exp_and_others")
    dst = os.path.join(dst_dir, "act_info.json")
    with open(dst, "w") as f:
        json.dump(info, f)
    os.environ["BASS_ACT_ROOT_JSON_PATH"] = dst

    import concourse.hw_specs as hw_specs
    tables = {
        ent["name"]: {
            mybir.ActivationFunctionType.from_pwp(v)
            for v in ent["act"].keys()
        }
        for ent in info["act_func_sets"]
    }
    bacc.get_activation_tables = lambda arch: tables
    hw_specs.get_activation_tables = lambda arch: tables

B, D, H, C = 131072, 128, 512, 64
NCORES = 8
TB = 512                # b-tile width
KH = H // 128           # 4 h-chunks of 128

F32 = mybir.dt.float32
F32R = mybir.dt.float32r
BF16 = mybir.dt.bfloat16

# ---- bf16 consts layout (columns in [128, NB] bf16 tensor) ----
W1_OFF = 0                       # 7 units * 512 = 3584
W2_OFF = W1_OFF + 7 * H          # 16 blocks (k*4+e) * 64 = 1024
G1_OFF = W2_OFF + 16 * 64        # 4 k * 32 cols (d0,-d0,0..)
G2A_OFF = G1_OFF + 4 * 32        # 4 k * 32 cols (dA,-dA,0..)
G2B_OFF = G2A_OFF + 4 * 32       # 4 k * 32 cols (0,0,dB,-dB,0..)
SA_OFF = G2B_OFF + 4 * 32        # 4 cols ones-select pair0
SB_OFF = SA_OFF + 4              # 4 cols ones-select pair1
ID_OFF = SB_OFF + 4              # 64 cols stacked identity
NB = ID_OFF + 64
# ---- fp32 consts layout ----
B1_OFF = 0                       # 28 cols (u*4+hb) L1 biases
EB_OFF = B1_OFF + 28             # 2 cols expert class biases (pair0, pair1)
GB_OFF = EB_OFF + 2              # 1 col gate-bias for E-exp (rows 0-35)
LB_OFF = GB_OFF + 1              # 1 col Ln bias (1.0 rows 0-63, 0.0 rows 64+)
NF = LB_OFF + 1
# ---- f32 M matrix [68, 256]: two 128-col blocks (pair0, pair1) ----


def _build_consts(gW1, gb1, gW2, gb2, eW1, eb1, eW2, eb2):
    cb = np.zeros((128, NB), dtype=np.float32)
    for u in range(3):
        cb[:, W1_OFF + u * H: W1_OFF + (u + 1) * H] = gW1[u]
    for e in range(4):
        cb[:, W1_OFF + (3 + e) * H: W1_OFF + (4 + e) * H] = eW1[e]
    for k in range(KH):
        for e in range(4):
            cb[:, W2_OFF + (k * 4 + e) * 64: W2_OFF + (k * 4 + e + 1) * 64] = \
                eW2[e, k * 128:(k + 1) * 128, :]
    v = gW2[:, :, 0] - gW2[:, :, 1]          # [3, 512]
    for k in range(KH):
        sl = slice(k * 128, (k + 1) * 128)
        cb[:, G1_OFF + k * 32 + 0] = v[0, sl]
        cb[:, G1_OFF + k * 32 + 1] = -v[0, sl]
        cb[:, G2A_OFF + k * 32 + 0] = v[1, sl]
        cb[:, G2A_OFF + k * 32 + 1] = -v[1, sl]
        cb[:, G2B_OFF + k * 32 + 2] = v[2, sl]
        cb[:, G2B_OFF + k * 32 + 3] = -v[2, sl]
    cb[0:64, SA_OFF + 0] = 1.0
    cb[64:128, SA_OFF + 1] = 1.0
    cb[0:64, SB_OFF + 2] = 1.0
    cb[64:128, SB_OFF + 3] = 1.0
    p = np.arange(128)
    cb[:, ID_OFF: ID_OFF + 64] = (p[:, None] % 64 == np.arange(64)[None, :])

    cf = np.zeros((128, NF), dtype=np.float32)
    b1 = np.concatenate([gb1, eb1], axis=0)  # [7, 512]
    for u in range(7):
        for hb in range(KH):
            cf[:, B1_OFF + u * 4 + hb] = b1[u, hb * 128:(hb + 1) * 128]
    cf[:64, EB_OFF + 0] = eb2[0]
    cf[64:, EB_OFF + 0] = eb2[1]
    cf[:64, EB_OFF + 1] = eb2[2]
    cf[64:, EB_OFF + 1] = eb2[3]
    db = gb2[:, 0] - gb2[:, 1]               # [3]
    # E-exp bias: E = exp(-(z + db)) = exp(-z + bias), bias rows:
    #   row 0: -db0 (z0), row 1: +db0 (-z0), rows 32/33: -+dbA, 34/35: -+dbB
    gB = np.zeros(128, dtype=np.float32)
    gB[0], gB[1] = -db[0], db[0]
    gB[32], gB[33] = -db[1], db[1]
    gB[34], gB[35] = -db[2], db[2]
    cf[:, GB_OFF] = gB
    lb = np.zeros(128, dtype=np.float32)
    lb[0:64] = 1.0                           # ln(E + 1) rows; ln(S + 0) rows
    cf[:, LB_OFF] = lb

    # M [68, 256] f32: delta = sum_q M[q, p] * V[q, b]
    M = np.zeros((68, 256), dtype=np.float32)
    # pair0 (cols 0-127): A1 block rows {0, 32, 64}; A2 block {0, 33, 65}
    M[0, 0:64] = -1.0; M[32, 0:64] = -1.0; M[64, 0:64] = -1.0
    M[0, 64:128] = -1.0; M[33, 64:128] = -1.0; M[65, 64:128] = -1.0
    # pair1 (cols 128-255): B1 {1, 34, 66}; B2 {1, 35, 67}
    M[1, 128:192] = -1.0; M[34, 128:192] = -1.0; M[66, 128:192] = -1.0
    M[1, 192:256] = -1.0; M[35, 192:256] = -1.0; M[67, 192:256] = -1.0

    zero_b1 = not (np.any(gb1) or np.any(eb1))
    return cb.astype(ml_dtypes.bfloat16), cf, M, zero_b1


def _build_nc(n_tiles, bc, zero_b1):
    _setup_act_tables()
    nc = bacc.Bacc("TRN2", target_bir_lowering=False)
    xt = nc.dram_tensor("xt", [D, bc], BF16, kind="ExternalInput")
    cbd = nc.dram_tensor("cb", [128, NB], BF16, kind="ExternalInput")
    cfd = nc.dram_tensor("cf", [128, NF], F32, kind="ExternalInput")
    md = nc.dram_tensor("md", [68, 256], F32R, kind="ExternalInput")
    outT = nc.dram_tensor("outT", [C, bc], F32, kind="ExternalOutput")

    AF = mybir.ActivationFunctionType
    OP = mybir.AluOpType

    # expert units first so L2 can start early; gate units last
    U_ORDER = [3, 4, 5, 6, 0, 1, 2]

    with tile.TileContext(nc) as tc:
        with (
            tc.tile_pool(name="singles", bufs=1) as singles,
            tc.tile_pool(name="xp", bufs=3) as xp,
            tc.tile_pool(name="hp", bufs=3) as hp,
            tc.tile_pool(name="ep", bufs=2) as ep,
            tc.tile_pool(name="vp", bufs=2) as vp,
            tc.tile_pool(name="cp", bufs=2) as cp,
            tc.tile_pool(name="pp", bufs=2) as pp,
            tc.tile_pool(name="op", bufs=2) as op_pool,
            tc.tile_pool(name="psL", bufs=2, space="PSUM") as psLp,
            tc.tile_pool(name="psE", bufs=1, space="PSUM") as psEp,
            tc.tile_pool(name="psG", bufs=1, space="PSUM") as psGp,
            tc.tile_pool(name="psT", bufs=1, space="PSUM") as psTp,
        ):
            cs = singles.tile([128, NB], BF16)
            nc.sync.dma_start(out=cs, in_=cbd[:, :])
            cf = singles.tile([128, NF], F32)
            nc.sync.dma_start(out=cf, in_=cfd[:, :])
            mm_ = singles.tile([68, 256], F32R)
            nc.sync.dma_start(out=mm_, in_=md[:, :])

            def w1_ap(u, hb):
                a = W1_OFF + u * H + hb * 128
                return cs[:, a: a + 128]

            def w2_ap(k, e):
                a = W2_OFF + (k * 4 + e) * 64
                return cs[:, a: a + 64]

            id2 = cs[:, ID_OFF: ID_OFF + 64]
            # B-pair order: experts u=3..6 first (pairs 0-7), gates u=0..2
            # (pairs 8-13). Pair i covers (u, hb2) with hb2 in {0, 2}.
            BPAIRS = [(u, hb2) for u in U_ORDER for hb2 in (0, 2)]
            # ACT/DVE split of L1 evacs (6 ACT, 8 DVE), spread out
            ACT_EVAC = {1, 4, 6, 8, 11, 13}

            def issue_E(st):
                # bcast MMs + exp(delta) + prods for a finished tile
                st["prods"] = []
                for pair in range(2):
                    psB = psTp.tile([128, TB], F32, tag="tail")
                    nc.tensor.matmul(
                        psB, mm_[:, pair * 128:(pair + 1) * 128],
                        st["V"][:, :], start=True, stop=True)
                    cbc = cp.tile([128, TB], BF16, tag=f"c{pair}")
                    nc.scalar.activation(cbc, psB, AF.Exp)
                    prod = pp.tile([128, TB], BF16, tag=f"p{pair}")
                    nc.vector.tensor_tensor(
                        prod, st["expc"][:, pair * TB:(pair + 1) * TB], cbc,
                        op=OP.mult)
                    st["prods"].append(prod)

            def issue_F(st):
                psO = psTp.tile([128, TB], F32, tag="tail")
                nc.tensor.matmul(psO[0:64, :], id2, st["prods"][0],
                                 start=True, stop=False)
                nc.tensor.matmul(psO[0:64, :], id2, st["prods"][1],
                                 start=False, stop=True)
                osb = op_pool.tile([64, TB], F32, tag="osb")
                nc.vector.tensor_copy(osb, psO[0:64, :])
                t0 = st["t"]
                nc.sync.dma_start(out=outT[:, t0 * TB:(t0 + 1) * TB], in_=osb)

            prev = None
            for t in range(n_tiles):
                xtile = xp.tile([D, TB], BF16, tag="x")
                nc.sync.dma_start(out=xtile, in_=xt[:, t * TB:(t + 1) * TB])
                st = {"t": t}
                hsb = {}

                def h_ap(u, k):
                    base = hsb[u, (k // 2) * 2]
                    j = k % 2
                    return base[:, j * TB:(j + 1) * TB]

                psE = None
                for i, (u, hb2) in enumerate(BPAIRS):
                    ps = psLp.tile([128, 2 * TB], F32, tag="l1")
                    nc.tensor.matmul(ps[:, 0:TB], w1_ap(u, hb2), xtile,
                                     start=True, stop=True)
                    nc.tensor.matmul(ps[:, TB:2 * TB], w1_ap(u, hb2 + 1),
                                     xtile, start=True, stop=True)
                    h = hp.tile([128, 2 * TB], BF16, tag=f"h{u}_{hb2}")
                    if zero_b1:
                        if i in ACT_EVAC:
                            nc.scalar.activation(h, ps, AF.Relu)
                        else:
                            nc.vector.tensor_scalar(
                                h, ps, 0.0, None, op0=OP.max)
                    else:
                        for j in range(2):
                            bap = cf[:, B1_OFF + u * 4 + hb2 + j:
                                     B1_OFF + u * 4 + hb2 + j + 1]
                            hj = h[:, j * TB:(j + 1) * TB]
                            pj = ps[:, j * TB:(j + 1) * TB]
                            if i in ACT_EVAC:
                                nc.scalar.activation(hj, pj, AF.Relu,
                                                     bias=bap)
                            else:
                                nc.vector.tensor_scalar(
                                    hj, pj, bap, 0.0, op0=OP.add, op1=OP.max)
                    hsb[u, hb2] = h

                    # ---- interleaved work keyed off B-pair index ----
                    if i == 1 and prev is not None:
                        issue_E(prev)
                    if i == 4 and prev is not None:
                        issue_F(prev)
                    if i == 5:
                        psE = psEp.tile([128, 2 * TB], F32, tag="e2")
                    if i in (5, 6, 8, 9):
                        # L2 expert k-groups: pair p, chunks (k, k+1)
                        p, kbase = {5: (0, 0), 6: (0, 2),
                                    8: (1, 0), 9: (1, 2)}[i]
                        for k in (kbase, kbase + 1):
                            sl = slice(p * TB, (p + 1) * TB)
                            nc.tensor.matmul(
                                psE[0:64, sl], w2_ap(k, 2 * p),
                                h_ap(3 + 2 * p, k), start=(k == 0),
                                stop=(k == KH - 1), tile_position=(0, 0))
                            nc.tensor.matmul(
                                psE[64:128, sl], w2_ap(k, 2 * p + 1),
                                h_ap(4 + 2 * p, k), start=(k == 0),
                                stop=(k == KH - 1), tile_position=(0, 64))
                    if i == 10:
                        expc = ep.tile([128, 2 * TB], BF16, tag="exp")
                        st["expc"] = expc
                        for pair in range(2):
                            sl = slice(pair * TB, (pair + 1) * TB)
                            nc.scalar.activation(
                                expc[:, sl], psE[:, sl], AF.Exp,
                                bias=cf[:, EB_OFF + pair: EB_OFF + pair + 1])
                    if i == 11:
                        psG = psGp.tile([68, TB], F32, tag="g")
                        for k in range(KH):
                            nc.tensor.matmul(
                                psG[0:32, :],
                                cs[:, G1_OFF + k * 32: G1_OFF + k * 32 + 32],
                                h_ap(0, k), start=(k == 0),
                                stop=(k == KH - 1), tile_position=(0, 0))
                    if i == 12:
                        for k in range(KH):
                            nc.tensor.matmul(
                                psG[32:64, :],
                                cs[:, G2A_OFF + k * 32: G2A_OFF + k * 32 + 32],
                                h_ap(1, k), start=(k == 0), stop=False,
                                tile_position=(0, 32))

                # ---- post-loop: G2B, S, E-exp, Ln ----
                for k in range(KH):
                    nc.tensor.matmul(
                        psG[32:64, :],
                        cs[:, G2B_OFF + k * 32: G2B_OFF + k * 32 + 32],
                        h_ap(2, k), start=False, stop=(k == KH - 1),
                        tile_position=(0, 32))
                nc.tensor.matmul(psG[64:68, :], cs[:, SA_OFF: SA_OFF + 4],
                                 expc[:, 0:TB], start=True, stop=False,
                                 tile_position=(0, 64))
                nc.tensor.matmul(psG[64:68, :], cs[:, SB_OFF: SB_OFF + 4],
                                 expc[:, TB:2 * TB], start=False, stop=True,
                                 tile_position=(0, 64))
                nc.scalar.activation(psG[0:64, :], psG[0:64, :], AF.Exp,
                                     bias=cf[0:64, GB_OFF: GB_OFF + 1],
                                     scale=-1.0)
                V = vp.tile([68, TB], F32R, tag="v")
                nc.scalar.activation(V, psG[0:68, :], AF.Ln,
                                     bias=cf[0:68, LB_OFF: LB_OFF + 1])
                st["V"] = V
                prev = st

            issue_E(prev)
            issue_F(prev)

    nc.compile()
    return nc


def kernel(x, gW1, gb1, gW2, gb2, eW1, eb1, eW2, eb2, _trace=False):
    x = np.asarray(x, dtype=np.float32)
    cb, cf, M, zero_b1 = _build_consts(
        np.asarray(gW1, np.float32), np.asarray(gb1, np.float32),
        np.asarray(gW2, np.float32), np.asarray(gb2, np.float32),
        np.asarray(eW1, np.float32), np.asarray(eb1, np.float32),
        np.asarray(eW2, np.float32), np.asarray(eb2, np.float32))
    n_rows = x.shape[0]
    bc = n_rows // NCORES
    n_tiles = bc // TB
    assert bc * NCORES == n_rows and n_tiles * TB == bc

    nc = _build_nc(n_tiles, bc, zero_b1)

    xs = x.reshape(NCORES, bc, D)
    in_maps = [
        {"xt": np.ascontiguousarray(xs[c].T).astype(ml_dtypes.bfloat16),
         "cb": cb, "cf": cf, "md": M}
        for c in range(NCORES)
    ]
    res = run_bass_kernel_spmd(nc, in_maps, core_ids=list(range(NCORES)),
                               trace=_trace)
    out = np.concatenate([r["outT"].T for r in res.results], axis=0)
    kernel.last_results = res
    return np.ascontiguousarray(out.astype(np.float32))
